# revision 20
# baseline (speedup 1.0000x reference)
"""Trainium2 Bass kernel for nn_MultiHeadAttention_41455024341166.

Reference computation (B=4, S=2048, M=2048, H=16, D=128, fp32):
    qkv = einsum('bsm,mthd->bsthd', x, Wqkv); q,k,v = qkv[:,:,0..2]
    q,k = rope_consecutive(q), rope_consecutive(k)
    ctx = causal_softmax(q @ k^T / sqrt(D)) @ v   (per b,h)
    out = ctx.reshape(B,S,H*D) @ Wo

Sharding: 8 cores = 4 batches x 2 head-groups (core c -> b=c//2, g=c%2,
heads [8g, 8g+8)). Attention is fully head-parallel; the output projection
produces partial sums over the head axis which a pairwise ReduceScatter
(bf16) combines (core 2b keeps rows [0,1024), core 2b+1 rows [1024,2048)).

v2 design (everything bf16 on the matmul paths, fp32 PSUM accumulate):
  - All intermediates SBUF-resident: xT (dies after A), qrotT/krotT
    [d,s] per head, v [s,hd], ctx [d,s] per head, wo. No DRAM roundtrip.
  - A-qk: per (head, q|k): W^T-block stationary @ xT moving -> [d,s];
    RoPE applied with strided-partition DVE views (pair-swap) + fp32
    cos/sin tables; result written straight into the resident q/k tiles.
  - A-v: sb-outer (key-block outer) so attention strips can interleave:
    v[sb] ready in key order; B(t) needs key blocks <= 4t+3 only.
  - B: per (strip t, head h): scoresT[j,i] blocks = krot-block stationary
    @ qrot-moving with the above-diagonal columns cut; exp fused into the
    PSUM evacuation (bf16 out); causal diagonal via multiplicative mask;
    denominators via ones-column matmuls accumulating in a [1,512] PSUM;
    ctxT += v-block @ expT. Consumer matmuls trail producers by 2 blocks
    so the tensor engine never waits on the Scalar-engine exp.
    Softmax denominators inverted with reciprocal_approx_fast (~5x the
    plain DVE reciprocal), broadcast by a K=1 ones matmul, and folded
    into the ctx PSUM evacuation. Normalization is deferred one head.
  - C: wo loaded once (recycling xT's SBUF ring slots), out chunks per
    strip lag B by one strip; partial outputs stored bf16 and combined
    with a pairwise bf16 ReduceScatter per strip (t=3 in 4 pieces so only
    the last ~0.5MB piece is exposed).
"""

import os
import sys
import types
import math

import numpy as np
import ml_dtypes

import concourse.bass as bass
import concourse.tile as tile
import concourse.mybir as mybir
from concourse.bass_utils import run_bass_kernel_spmd

F32 = mybir.dt.float32
F32R = mybir.dt.float32r
BF16 = mybir.dt.bfloat16
NP_BF16 = np.dtype(ml_dtypes.bfloat16)

B, S, M, H, D = 4, 2048, 2048, 16, 128
HL = H // 2              # heads per core
HD = HL * D              # 1024
SCALE = 1.0 / math.sqrt(D)
MIN_WINDOW, MAX_WINDOW = 1.0, 10000.0


# ---------------------------------------------------------------------------
# Workarounds for the trimmed walrus/axon stack in this container.
# ---------------------------------------------------------------------------

_WSPLIT_N = [0]


def _split_excess_waits(nc):
    """walrus here rejects instructions carrying more sync-waits than slots
    (1; EventSemaphore: 2). Hoist excess waits onto EventSemaphore carriers
    inserted before the offender on the same engine stream. Safe: Tile emits
    one linearized order where every wait's producer precedes its consumer."""
    for fn in nc.m.functions:
        for bb in fn.blocks:
            changed = False
            new_list = []
            for inst in bb.instructions:
                si = inst.sync_info
                waits = list(si.on_wait) if si is not None else []
                cap = 2 if isinstance(inst, mybir.InstEventSemaphore) else 1
                if len(waits) > cap:
                    keep, excess = waits[-cap:], waits[:-cap]
                    for i in range(0, len(excess), 2):
                        _WSPLIT_N[0] += 1
                        new_list.append(mybir.InstEventSemaphore(
                            name=f"wsplit-{_WSPLIT_N[0]}", ins=[], outs=[],
                            engine=inst.engine,
                            sync_info=mybir.SyncInfo(on_wait=excess[i:i + 2],
                                                     on_update=[])))
                    si.on_wait = keep
                    changed = True
                new_list.append(inst)
            if changed:
                bb.instructions = new_list


def _register_ntff_hook():
    """antenv.axon_hooks is absent in this image, so boot skipped registering
    the NTFF profiling hook; recreate it so trace=True works."""
    if "antenv.axon_hooks" in sys.modules:
        return
    try:
        import antenv as _antenv
        m = types.ModuleType("antenv.axon_hooks")
        m._hook = None
        m.set_axon_ntff_profile_hook = lambda h, _m=m: setattr(_m, "_hook", h)
        m.get_axon_ntff_profile_hook = lambda _m=m: _m._hook
        sys.modules["antenv.axon_hooks"] = m
        _antenv.axon_hooks = m
        from trn_agent_boot.trn_boot import _ntff_profile_via_ctypes
        m.set_axon_ntff_profile_hook(
            _ntff_profile_via_ctypes('/opt/axon/libaxon_pjrt.so'))
    except Exception:
        pass


_register_ntff_hook()


# ---------------------------------------------------------------------------
# Kernel builder (per-core SPMD program)
# ---------------------------------------------------------------------------

def _blocked_dma(eng, dst_ap, dram_full, c0, c1):
    """One DMA moving cols [c0,c1) of a [R, C] DRAM tensor into a
    [128, (R//128)*(c1-c0)] SBUF tile whose column block a holds source rows
    [a*128, (a+1)*128)."""
    src = dram_full.rearrange("(a p) c -> p a c", p=128)[:, :, c0:c1]
    dst = dst_ap.rearrange("p (a c) -> p a c", c=c1 - c0)
    eng.dma_start(dst, src)


def build_kernel():
    nc = bass.Bass("TRN2", target_bir_lowering=False, num_devices=8)

    xt = nc.dram_tensor("xt", [M, S], BF16, kind="ExternalInput")       # x[b].T
    wq = nc.dram_tensor("wq", [M, HD], BF16, kind="ExternalInput")
    wk = nc.dram_tensor("wk", [M, HD], BF16, kind="ExternalInput")
    wv = nc.dram_tensor("wv", [M, HD], BF16, kind="ExternalInput")
    wo = nc.dram_tensor("wo", [HD, M], BF16, kind="ExternalInput")
    cosT = nc.dram_tensor("cosT", [D, S], BF16, kind="ExternalInput")
    sinT = nc.dram_tensor("sinT", [D, S], BF16, kind="ExternalInput")   # sign-folded
    mask128 = nc.dram_tensor("mask128", [128, 128], BF16, kind="ExternalInput")
    # RS quarters: y[t] = out[b, t*512 + half*256 : +256, :] for this core's half
    y = nc.dram_tensor("y", [4, 256, M], BF16, kind="ExternalOutput")

    with nc.allow_low_precision(reason="bf16 matmul kernel"), \
         tile.TileContext(nc) as tc:
        with tc.tile_pool(name="dram", bufs=1, space="DRAM") as dram:
            outp_t = [dram.tile([512, M], BF16, name=f"outp{i}") for i in range(3)]
            rs_t = [dram.tile([256, M], BF16, name=f"rst{i}") for i in range(3)]
            outp3 = [dram.tile([512, 1536], BF16, name="outp3a"),
                     dram.tile([512, 512], BF16, name="outp3b")]
            rs3 = [dram.tile([256, 1536], BF16, name="rst3a"),
                   dram.tile([256, 512], BF16, name="rst3b")]

            from contextlib import ExitStack
            with ExitStack() as _es:
                tabs = _es.enter_context(tc.tile_pool(name="tabs", bufs=1))
                xw = _es.enter_context(tc.tile_pool(name="xw", bufs=16))
                qkp = _es.enter_context(tc.tile_pool(name="qk", bufs=1))
                vp = _es.enter_context(tc.tile_pool(name="vp", bufs=1))
                wp = _es.enter_context(tc.tile_pool(name="wblk", bufs=2))
                wvp = _es.enter_context(tc.tile_pool(name="wvp", bufs=1))
                w512 = _es.enter_context(tc.tile_pool(name="w512", bufs=5))
                qsp = _es.enter_context(tc.tile_pool(name="qs", bufs=2))
                smp = _es.enter_context(tc.tile_pool(name="sm", bufs=2))
                cop = _es.enter_context(tc.tile_pool(name="co", bufs=3))
                from contextlib import ExitStack as _ES2
                _esA = _es.enter_context(_ES2())
                psA = _esA.enter_context(tc.tile_pool(name="psA", bufs=4, space="PSUM"))
                # ---- first weight blocks ahead of everything (gpsimd queue) ----
                wblk_pre = {}
                for h0, qk0, wt0 in ((0, 0, wq), (0, 1, wk)):
                    wb = wp.tile([128, 16 * 128], BF16,
                                 name=f"wpre{h0}{qk0}", tag="wblk")
                    _blocked_dma(nc.gpsimd, wb[:], wt0[:], h0 * 128, (h0 + 1) * 128)
                    wblk_pre[(h0, qk0)] = wb

                # ---- tables / tiny constants (gpsimd queue) ----
                cos_sb = tabs.tile([128, S], BF16)
                nc.gpsimd.dma_start(cos_sb[:], cosT[:])
                sin_sb = tabs.tile([128, S], BF16)
                nc.gpsimd.dma_start(sin_sb[:], sinT[:])
                mask_sb = tabs.tile([128, 128], BF16)
                nc.gpsimd.dma_start(mask_sb[:], mask128[:])
                ones128 = tabs.tile([128, 128], BF16)
                nc.vector.memset(ones128[:], 1.0)

                # ---- resident xT: 16 ring slots [128, S] bf16 (2MB ea /4) ----
                xts = []
                for q4 in range(16):
                    xti = xw.tile([128, S], BF16, name=f"xt{q4}", tag="xw")
                    eng = nc.sync if q4 % 2 == 0 else nc.scalar
                    eng.dma_start(
                        xti[:],
                        xt.rearrange("(a p) c -> p a c", p=128)[:, q4, :])
                    xts.append(xti)

                # ---- resident outputs of phase A ----
                # ctx_sb aliases qrot: strip t of qrot[h] is dead once
                # B(h,t)'s score matmuls finish, exactly when ctx strip t
                # is written by the deferred flush.
                qrot = [qkp.tile([128, S], BF16, name=f"qr{h}") for h in range(HL)]
                krot = [qkp.tile([128, S], BF16, name=f"kr{h}") for h in range(HL)]
                vsb = [vp.tile([128, HD], BF16, name=f"v{sb}") for sb in range(16)]
                ctx_sb = qrot

                # ======== Phase A-qk: qT,kT projections + RoPE ========
                def emit_aqk(h, qk, wt, outt):
                    if (h, qk) in wblk_pre:
                        wblk = wblk_pre.pop((h, qk))
                    else:
                        wblk = wp.tile([128, 16 * 128], BF16,
                                       name=f"wblk{h}{qk}", tag="wblk")
                        _blocked_dma(nc.sync, wblk[:], wt[:],
                                     h * 128, (h + 1) * 128)
                    for t in range(4):
                        ps = psA.tile([128, 512], F32,
                                      name=f"psq{h}{qk}{t}", tag="acc")
                        for mt in range(16):
                            nc.tensor.matmul(
                                ps[:],
                                wblk[:, mt * 128:(mt + 1) * 128],
                                xts[mt][:, t * 512:(t + 1) * 512],
                                start=(mt == 0), stop=(mt == 15))
                        q_sb = w512.tile([128, 512], BF16,
                                         name=f"q{h}{qk}{t}", tag="w512")
                        nc.scalar.copy(q_sb[:], ps[:])
                        cs = slice(t * 512, (t + 1) * 512)
                        t1 = w512.tile([128, 512], BF16,
                                       name=f"t1{h}{qk}{t}", tag="w512")
                        nc.vector.tensor_mul(t1[:], q_sb[:], cos_sb[:, cs])
                        qs = qsp.tile([128, 512], BF16,
                                      name=f"qs{h}{qk}{t}", tag="qs")
                        nc.gpsimd.dma_start(qs[0:127:2, :], q_sb[1:128:2, :])
                        nc.gpsimd.dma_start(qs[1:128:2, :], q_sb[0:127:2, :])
                        t2 = w512.tile([128, 512], BF16,
                                       name=f"t2{h}{qk}{t}", tag="w512")
                        nc.vector.tensor_mul(t2[:], qs[:], sin_sb[:, cs])
                        nc.vector.tensor_add(outt[h][:, cs], t1[:], t2[:])

                with nc.named_scope("A_qk"):
                    for h in range(HL):
                        emit_aqk(h, 0, wq, qrot)
                        emit_aqk(h, 1, wk, krot)

                # ======== Phase A-v (ds-outer, single wv buffer) ========
                wvs_cur = [None]

                def load_wvs(ds):
                    wvs = wvp.tile([128, 16 * 512], BF16,
                                   name=f"wvs{ds}", tag="wvs")
                    _blocked_dma(nc.gpsimd, wvs[:], wv[:],
                                 ds * 512, (ds + 1) * 512)
                    wvs_cur[0] = wvs

                def emit_av(ds, sb):
                    with nc.named_scope("A_v"):
                        ps = psA.tile([128, 512], F32,
                                      name=f"psv{sb}{ds}", tag="acc")
                        for mt in range(16):
                            nc.tensor.matmul(
                                ps[:],
                                xts[mt][:, sb * 128:(sb + 1) * 128],
                                wvs_cur[0][:, mt * 512:(mt + 1) * 512],
                                start=(mt == 0), stop=(mt == 15))
                        nc.scalar.copy(
                            vsb[sb][:, ds * 512:(ds + 1) * 512], ps[:])

                load_wvs(0)
                for sb in range(16):
                    emit_av(0, sb)
                load_wvs(1)
                for sb in range(16):
                    emit_av(1, sb)
                _esA.close()
                psS = _es.enter_context(tc.tile_pool(name="psS", bufs=3, space="PSUM"))
                psC = _es.enter_context(tc.tile_pool(name="psC", bufs=3, space="PSUM"))
                psB = _es.enter_context(tc.tile_pool(name="psB", bufs=2, space="PSUM"))

                # wo ring-reuses xts slots 0..7: DMA waits until A-v's last read
                wos = []
                for a in range(8):
                    wt_ = xw.tile([128, S], BF16, name=f"wo{a}", tag="xw")
                    nc.sync.dma_start(wt_[:], wo[a * 128:(a + 1) * 128, :])
                    wos.append(wt_)

                # strip-deferred normalization state: (pc, rec, h, t)
                pending = []

                def flush_pending():
                    while pending:
                        pcp_, recp_, hp_, tp2_ = pending.pop(0)
                        nc.vector.tensor_mul(
                            ctx_sb[hp_][:, tp2_ * 512:(tp2_ + 1) * 512],
                            pcp_[:], recp_[:])

                def emit_b(h, t):
                    flush_pending()
                    njt = 4 * t + 4
                    qr = qrot[h][:, t * 512:(t + 1) * 512]
                    pc = psC.tile([128, 512], F32, name=f"pc{h}{t}", tag="pc")
                    pm = psB.tile([128, 512], F32, name=f"pm{h}{t}", tag="pmb")
                    exs = []

                    def emit_front(jt):
                        # scoresT block + exp into SBUF (+ diagonal mask)
                        cut = 128 * (jt - 4 * t) if jt >= 4 * t else 0
                        pss = psS.tile([128, 512], F32,
                                       name=f"pss{h}{t}{jt}", tag="pss")
                        nc.tensor.matmul(pss[:, cut:512],
                                         krot[h][:, jt * 128:(jt + 1) * 128],
                                         qr[:, cut:512], start=True, stop=True,
                                         skip_group_check=True)
                        ex = w512.tile([128, 512], BF16,
                                       name=f"ex{h}{t}{jt}", tag="w512")
                        nc.scalar.activation(
                            ex[:, cut:512], pss[:, cut:512],
                            mybir.ActivationFunctionType.Exp, scale=SCALE)
                        if jt >= 4 * t:
                            nc.vector.tensor_mul(
                                ex[:, cut:cut + 128],
                                ex[:, cut:cut + 128], mask_sb[:])
                        exs.append((ex, cut))

                    def emit_back(jt):
                        ex, cut = exs[jt]
                        nc.tensor.matmul(pm[:, cut:512], ones128[:],
                                         ex[:, cut:512],
                                         start=(jt == 0), stop=(jt == njt - 1),
                                         skip_group_check=True)
                        nc.tensor.matmul(pc[:, cut:512],
                                         vsb[jt][:, h * 128:(h + 1) * 128],
                                         ex[:, cut:512],
                                         start=(jt == 0), stop=(jt == njt - 1),
                                         skip_group_check=True)

                    with nc.named_scope("B_attn"):
                        emit_front(0)
                        emit_front(1)
                        emit_front(2)
                        for jt in range(3, njt):
                            emit_front(jt)
                            emit_back(jt - 3)
                        emit_back(njt - 3)
                        emit_back(njt - 2)
                        emit_back(njt - 1)
                        rec = smp.tile([128, 512], F32R,
                                       name=f"rec{h}{t}", tag="rec")
                        nc.vector.reciprocal(rec[:], pm[:])
                        pending.append((pc, rec, h, t))

                def emit_c(t):
                    with nc.named_scope(f"C_out{t}"):
                        for ms in range(4):
                            for sbl in range(4):
                                sb = 4 * t + sbl
                                po = psC.tile([128, 512], F32,
                                              name=f"po{t}{sbl}{ms}", tag="pc")
                                for ht in range(HL):
                                    nc.tensor.matmul(
                                        po[:],
                                        ctx_sb[ht][:, sb * 128:(sb + 1) * 128],
                                        wos[ht][:, ms * 512:(ms + 1) * 512],
                                        start=(ht == 0), stop=(ht == HL - 1))
                                ot = cop.tile([128, 512], BF16,
                                              name=f"ot{t}{sbl}{ms}", tag="ot")
                                nc.scalar.copy(ot[:], po[:])
                                if t == 3:
                                    piece, mo = (0, ms) if ms < 3 else (1, 0)
                                    dst = outp3[piece][sbl * 128:(sbl + 1) * 128,
                                                       mo * 512:(mo + 1) * 512]
                                else:
                                    dst = outp_t[t][sbl * 128:(sbl + 1) * 128,
                                                    ms * 512:(ms + 1) * 512]
                                nc.sync.dma_start(dst, ot[:])
                            if t == 3 and ms in (2, 3):
                                piece = 0 if ms == 2 else 1
                                c0 = 0 if ms == 2 else 1536
                                nc.gpsimd.collective_compute(
                                    "ReduceScatter", mybir.AluOpType.add,
                                    replica_groups=[[0, 1], [2, 3], [4, 5], [6, 7]],
                                    ins=[outp3[piece][:]], outs=[rs3[piece][:]])
                                nc.gpsimd.dma_start(
                                    y[3][:, c0:c0 + rs3[piece].shape[1]],
                                    rs3[piece][:])
                        if t < 3:
                            nc.gpsimd.collective_compute(
                                "ReduceScatter", mybir.AluOpType.add,
                                replica_groups=[[0, 1], [2, 3], [4, 5], [6, 7]],
                                ins=[outp_t[t][:]], outs=[rs_t[t][:]])
                            nc.gpsimd.dma_start(y[t], rs_t[t][:])

                # B strips with A-v blocks interleaved; C lags B by one strip
                for t in range(4):
                    for h in range(HL):
                        emit_b(h, t)
                    if t == 1:
                        flush_pending()
                        emit_c(0)
                    elif t == 2:
                        flush_pending()
                        emit_c(1)
                        emit_c(2)
                flush_pending()
                emit_c(3)

    _split_excess_waits(nc)
    return nc


# ---------------------------------------------------------------------------
# Host-side input prep / sharding
# ---------------------------------------------------------------------------

def _rope_tables():
    half = D // 2
    fraction = 2.0 * np.arange(half, dtype=np.float64) / D
    ts = MIN_WINDOW * (MAX_WINDOW / MIN_WINDOW) ** fraction
    ts = np.repeat(ts, 2)                              # [D]
    pos = np.arange(S, dtype=np.float64)
    sinusoid = pos[None, :] / ts[:, None]              # [D, S]
    cos = np.cos(sinusoid).astype(NP_BF16)
    sign = np.where(np.arange(D) % 2 == 1, 1.0, -1.0)
    sin = (np.sin(sinusoid) * sign[:, None]).astype(NP_BF16)
    return cos, sin


def _mask128():
    jj = np.arange(128)[:, None]
    ii = np.arange(128)[None, :]
    return (jj <= ii).astype(NP_BF16)



_CACHED = {}


def kernel(x, Wqkv, Wo):
    x = np.asarray(x, dtype=np.float32)
    Wqkv = np.asarray(Wqkv, dtype=np.float32)
    Wo = np.asarray(Wo, dtype=np.float32)

    cos, sin = _rope_tables()
    m128 = _mask128()

    in_maps = []
    for c in range(8):
        b, g = c // 2, c % 2
        hs = slice(g * HL, (g + 1) * HL)
        in_maps.append({
            "xt": np.ascontiguousarray(x[b].T).astype(NP_BF16),
            "wq": np.ascontiguousarray(
                Wqkv[:, 0, hs, :].reshape(M, HD)).astype(NP_BF16),
            "wk": np.ascontiguousarray(
                Wqkv[:, 1, hs, :].reshape(M, HD)).astype(NP_BF16),
            "wv": np.ascontiguousarray(
                Wqkv[:, 2, hs, :].reshape(M, HD)).astype(NP_BF16),
            "wo": np.ascontiguousarray(
                Wo[g * HD:(g + 1) * HD, :]).astype(NP_BF16),
            "cosT": cos, "sinT": sin, "mask128": m128,
        })

    if "nc" not in _CACHED:
        _CACHED["nc"] = build_kernel()
    nc = _CACHED["nc"]

    res = run_bass_kernel_spmd(nc, in_maps, core_ids=list(range(8)),
                               trace=os.environ.get("MHA_KERNEL_TRACE", "0") == "1")
    _CACHED["last_results"] = res

    out = np.empty((B, S, M), dtype=np.float32)
    for b in range(B):
        for half, r in ((0, res.results[2 * b]["y"]),
                        (256, res.results[2 * b + 1]["y"])):
            for t in range(4):
                out[b, t * 512 + half: t * 512 + half + 256] = \
                    np.asarray(r[t]).astype(np.float32)
    return out


if __name__ == "__main__":
    rng = np.random.default_rng(0)
    x = rng.standard_normal((B, S, M), dtype=np.float32)
    Wqkv = (rng.standard_normal((M, 3, H, D), dtype=np.float32) / math.sqrt(M)).astype(np.float32)
    Wo = (rng.standard_normal((H * D, M), dtype=np.float32) / math.sqrt(H * D)).astype(np.float32)
    out = kernel(x=x, Wqkv=Wqkv, Wo=Wo)
    print("kernel ran, out shape", out.shape, "mean", float(np.abs(out).mean()))


# revision 21
# speedup vs baseline: 1.0003x; 1.0003x over previous
"""Trainium2 Bass kernel for nn_MultiHeadAttention_41455024341166.

Reference computation (B=4, S=2048, M=2048, H=16, D=128, fp32):
    qkv = einsum('bsm,mthd->bsthd', x, Wqkv); q,k,v = qkv[:,:,0..2]
    q,k = rope_consecutive(q), rope_consecutive(k)
    ctx = causal_softmax(q @ k^T / sqrt(D)) @ v   (per b,h)
    out = ctx.reshape(B,S,H*D) @ Wo

Sharding: 8 cores = 4 batches x 2 head-groups (core c -> b=c//2, g=c%2,
heads [8g, 8g+8)). Attention is fully head-parallel; the output projection
produces partial sums over the head axis which a pairwise ReduceScatter
(bf16) combines (core 2b keeps rows [0,1024), core 2b+1 rows [1024,2048)).

v2 design (everything bf16 on the matmul paths, fp32 PSUM accumulate):
  - All intermediates SBUF-resident: xT (dies after A), qrotT/krotT
    [d,s] per head, v [s,hd], ctx [d,s] per head, wo. No DRAM roundtrip.
  - A-qk: per (head, q|k): W^T-block stationary @ xT moving -> [d,s];
    RoPE applied with strided-partition DVE views (pair-swap) + fp32
    cos/sin tables; result written straight into the resident q/k tiles.
  - A-v: sb-outer (key-block outer) so attention strips can interleave:
    v[sb] ready in key order; B(t) needs key blocks <= 4t+3 only.
  - B: per (strip t, head h): scoresT[j,i] blocks = krot-block stationary
    @ qrot-moving with the above-diagonal columns cut; exp fused into the
    PSUM evacuation (bf16 out); causal diagonal via multiplicative mask;
    denominators via ones-column matmuls accumulating in a [1,512] PSUM;
    ctxT += v-block @ expT. Consumer matmuls trail producers by 2 blocks
    so the tensor engine never waits on the Scalar-engine exp.
    Softmax denominators inverted with reciprocal_approx_fast (~5x the
    plain DVE reciprocal), broadcast by a K=1 ones matmul, and folded
    into the ctx PSUM evacuation. Normalization is deferred one head.
  - C: wo loaded once (recycling xT's SBUF ring slots), out chunks per
    strip lag B by one strip; partial outputs stored bf16 and combined
    with a pairwise bf16 ReduceScatter per strip (t=3 in 4 pieces so only
    the last ~0.5MB piece is exposed).
"""

import os
import sys
import types
import math

import numpy as np
import ml_dtypes

import concourse.bass as bass
import concourse.tile as tile
import concourse.mybir as mybir
from concourse.bass_utils import run_bass_kernel_spmd

F32 = mybir.dt.float32
F32R = mybir.dt.float32r
BF16 = mybir.dt.bfloat16
NP_BF16 = np.dtype(ml_dtypes.bfloat16)

B, S, M, H, D = 4, 2048, 2048, 16, 128
HL = H // 2              # heads per core
HD = HL * D              # 1024
SCALE = 1.0 / math.sqrt(D)
MIN_WINDOW, MAX_WINDOW = 1.0, 10000.0


# ---------------------------------------------------------------------------
# Workarounds for the trimmed walrus/axon stack in this container.
# ---------------------------------------------------------------------------

_WSPLIT_N = [0]


def _split_excess_waits(nc):
    """walrus here rejects instructions carrying more sync-waits than slots
    (1; EventSemaphore: 2). Hoist excess waits onto EventSemaphore carriers
    inserted before the offender on the same engine stream. Safe: Tile emits
    one linearized order where every wait's producer precedes its consumer."""
    for fn in nc.m.functions:
        for bb in fn.blocks:
            changed = False
            new_list = []
            for inst in bb.instructions:
                si = inst.sync_info
                waits = list(si.on_wait) if si is not None else []
                cap = 2 if isinstance(inst, mybir.InstEventSemaphore) else 1
                if len(waits) > cap:
                    keep, excess = waits[-cap:], waits[:-cap]
                    for i in range(0, len(excess), 2):
                        _WSPLIT_N[0] += 1
                        new_list.append(mybir.InstEventSemaphore(
                            name=f"wsplit-{_WSPLIT_N[0]}", ins=[], outs=[],
                            engine=inst.engine,
                            sync_info=mybir.SyncInfo(on_wait=excess[i:i + 2],
                                                     on_update=[])))
                    si.on_wait = keep
                    changed = True
                new_list.append(inst)
            if changed:
                bb.instructions = new_list


def _register_ntff_hook():
    """antenv.axon_hooks is absent in this image, so boot skipped registering
    the NTFF profiling hook; recreate it so trace=True works."""
    if "antenv.axon_hooks" in sys.modules:
        return
    try:
        import antenv as _antenv
        m = types.ModuleType("antenv.axon_hooks")
        m._hook = None
        m.set_axon_ntff_profile_hook = lambda h, _m=m: setattr(_m, "_hook", h)
        m.get_axon_ntff_profile_hook = lambda _m=m: _m._hook
        sys.modules["antenv.axon_hooks"] = m
        _antenv.axon_hooks = m
        from trn_agent_boot.trn_boot import _ntff_profile_via_ctypes
        m.set_axon_ntff_profile_hook(
            _ntff_profile_via_ctypes('/opt/axon/libaxon_pjrt.so'))
    except Exception:
        pass


_register_ntff_hook()


# ---------------------------------------------------------------------------
# Kernel builder (per-core SPMD program)
# ---------------------------------------------------------------------------

def _blocked_dma(eng, dst_ap, dram_full, c0, c1):
    """One DMA moving cols [c0,c1) of a [R, C] DRAM tensor into a
    [128, (R//128)*(c1-c0)] SBUF tile whose column block a holds source rows
    [a*128, (a+1)*128)."""
    src = dram_full.rearrange("(a p) c -> p a c", p=128)[:, :, c0:c1]
    dst = dst_ap.rearrange("p (a c) -> p a c", c=c1 - c0)
    eng.dma_start(dst, src)


def build_kernel():
    nc = bass.Bass("TRN2", target_bir_lowering=False, num_devices=8)

    xt = nc.dram_tensor("xt", [M, S], BF16, kind="ExternalInput")       # x[b].T
    wq = nc.dram_tensor("wq", [M, HD], BF16, kind="ExternalInput")
    wk = nc.dram_tensor("wk", [M, HD], BF16, kind="ExternalInput")
    wv = nc.dram_tensor("wv", [M, HD], BF16, kind="ExternalInput")
    wo = nc.dram_tensor("wo", [HD, M], BF16, kind="ExternalInput")
    cosT = nc.dram_tensor("cosT", [D, S], BF16, kind="ExternalInput")
    sinT = nc.dram_tensor("sinT", [D, S], BF16, kind="ExternalInput")   # sign-folded
    mask128 = nc.dram_tensor("mask128", [128, 128], BF16, kind="ExternalInput")
    # RS quarters: y[t] = out[b, t*512 + half*256 : +256, :] for this core's half
    y = nc.dram_tensor("y", [4, 256, M], BF16, kind="ExternalOutput")

    with nc.allow_low_precision(reason="bf16 matmul kernel"), \
         tile.TileContext(nc) as tc:
        with tc.tile_pool(name="dram", bufs=1, space="DRAM") as dram:
            outp_t = [dram.tile([512, M], BF16, name=f"outp{i}") for i in range(3)]
            rs_t = [dram.tile([256, M], BF16, name=f"rst{i}") for i in range(3)]
            outp3 = [dram.tile([512, 1024], BF16, name="outp3a"),
                     dram.tile([512, 1024], BF16, name="outp3b")]
            rs3 = [dram.tile([256, 1024], BF16, name="rst3a"),
                   dram.tile([256, 1024], BF16, name="rst3b")]

            from contextlib import ExitStack
            with ExitStack() as _es:
                tabs = _es.enter_context(tc.tile_pool(name="tabs", bufs=1))
                xw = _es.enter_context(tc.tile_pool(name="xw", bufs=16))
                qkp = _es.enter_context(tc.tile_pool(name="qk", bufs=1))
                vp = _es.enter_context(tc.tile_pool(name="vp", bufs=1))
                wp = _es.enter_context(tc.tile_pool(name="wblk", bufs=2))
                wvp = _es.enter_context(tc.tile_pool(name="wvp", bufs=1))
                w512 = _es.enter_context(tc.tile_pool(name="w512", bufs=5))
                qsp = _es.enter_context(tc.tile_pool(name="qs", bufs=2))
                smp = _es.enter_context(tc.tile_pool(name="sm", bufs=2))
                cop = _es.enter_context(tc.tile_pool(name="co", bufs=3))
                from contextlib import ExitStack as _ES2
                _esA = _es.enter_context(_ES2())
                psA = _esA.enter_context(tc.tile_pool(name="psA", bufs=4, space="PSUM"))
                # ---- first weight blocks ahead of everything (gpsimd queue) ----
                wblk_pre = {}
                for h0, qk0, wt0 in ((0, 0, wq), (0, 1, wk)):
                    wb = wp.tile([128, 16 * 128], BF16,
                                 name=f"wpre{h0}{qk0}", tag="wblk")
                    _blocked_dma(nc.gpsimd, wb[:], wt0[:], h0 * 128, (h0 + 1) * 128)
                    wblk_pre[(h0, qk0)] = wb

                # ---- tables / tiny constants (gpsimd queue) ----
                cos_sb = tabs.tile([128, S], BF16)
                nc.gpsimd.dma_start(cos_sb[:], cosT[:])
                sin_sb = tabs.tile([128, S], BF16)
                nc.gpsimd.dma_start(sin_sb[:], sinT[:])
                mask_sb = tabs.tile([128, 128], BF16)
                nc.gpsimd.dma_start(mask_sb[:], mask128[:])
                ones128 = tabs.tile([128, 128], BF16)
                nc.vector.memset(ones128[:], 1.0)

                # ---- resident xT: 16 ring slots [128, S] bf16 (2MB ea /4) ----
                xts = []
                for q4 in range(16):
                    xti = xw.tile([128, S], BF16, name=f"xt{q4}", tag="xw")
                    eng = nc.sync if q4 % 2 == 0 else nc.scalar
                    eng.dma_start(
                        xti[:],
                        xt.rearrange("(a p) c -> p a c", p=128)[:, q4, :])
                    xts.append(xti)

                # ---- resident outputs of phase A ----
                # ctx_sb aliases qrot: strip t of qrot[h] is dead once
                # B(h,t)'s score matmuls finish, exactly when ctx strip t
                # is written by the deferred flush.
                qrot = [qkp.tile([128, S], BF16, name=f"qr{h}") for h in range(HL)]
                krot = [qkp.tile([128, S], BF16, name=f"kr{h}") for h in range(HL)]
                vsb = [vp.tile([128, HD], BF16, name=f"v{sb}") for sb in range(16)]
                ctx_sb = qrot

                # ======== Phase A-qk: qT,kT projections + RoPE ========
                def emit_aqk(h, qk, wt, outt):
                    if (h, qk) in wblk_pre:
                        wblk = wblk_pre.pop((h, qk))
                    else:
                        wblk = wp.tile([128, 16 * 128], BF16,
                                       name=f"wblk{h}{qk}", tag="wblk")
                        _blocked_dma(nc.sync, wblk[:], wt[:],
                                     h * 128, (h + 1) * 128)
                    for t in range(4):
                        ps = psA.tile([128, 512], F32,
                                      name=f"psq{h}{qk}{t}", tag="acc")
                        for mt in range(16):
                            nc.tensor.matmul(
                                ps[:],
                                wblk[:, mt * 128:(mt + 1) * 128],
                                xts[mt][:, t * 512:(t + 1) * 512],
                                start=(mt == 0), stop=(mt == 15))
                        q_sb = w512.tile([128, 512], BF16,
                                         name=f"q{h}{qk}{t}", tag="w512")
                        nc.scalar.copy(q_sb[:], ps[:])
                        cs = slice(t * 512, (t + 1) * 512)
                        t1 = w512.tile([128, 512], BF16,
                                       name=f"t1{h}{qk}{t}", tag="w512")
                        nc.vector.tensor_mul(t1[:], q_sb[:], cos_sb[:, cs])
                        qs = qsp.tile([128, 512], BF16,
                                      name=f"qs{h}{qk}{t}", tag="qs")
                        nc.gpsimd.dma_start(qs[0:127:2, :], q_sb[1:128:2, :])
                        nc.gpsimd.dma_start(qs[1:128:2, :], q_sb[0:127:2, :])
                        t2 = w512.tile([128, 512], BF16,
                                       name=f"t2{h}{qk}{t}", tag="w512")
                        nc.vector.tensor_mul(t2[:], qs[:], sin_sb[:, cs])
                        nc.vector.tensor_add(outt[h][:, cs], t1[:], t2[:])

                with nc.named_scope("A_qk"):
                    for h in range(HL):
                        emit_aqk(h, 0, wq, qrot)
                        emit_aqk(h, 1, wk, krot)

                # ======== Phase A-v (ds-outer, single wv buffer) ========
                wvs_cur = [None]

                def load_wvs(ds):
                    wvs = wvp.tile([128, 16 * 512], BF16,
                                   name=f"wvs{ds}", tag="wvs")
                    src_v = wv.rearrange("(a p) c -> p a c", p=128)
                    dst_v = wvs[:].rearrange("p (a c) -> p a c", c=512)
                    for ch in range(4):
                        nc.gpsimd.dma_start(
                            dst_v[:, ch * 4:(ch + 1) * 4, :],
                            src_v[:, ch * 4:(ch + 1) * 4,
                                  ds * 512:(ds + 1) * 512])
                    wvs_cur[0] = wvs

                def emit_av(ds, sb):
                    with nc.named_scope("A_v"):
                        ps = psA.tile([128, 512], F32,
                                      name=f"psv{sb}{ds}", tag="acc")
                        for mt in range(16):
                            nc.tensor.matmul(
                                ps[:],
                                xts[mt][:, sb * 128:(sb + 1) * 128],
                                wvs_cur[0][:, mt * 512:(mt + 1) * 512],
                                start=(mt == 0), stop=(mt == 15))
                        nc.scalar.copy(
                            vsb[sb][:, ds * 512:(ds + 1) * 512], ps[:])

                load_wvs(0)
                for sb in range(16):
                    emit_av(0, sb)
                load_wvs(1)
                for sb in range(16):
                    emit_av(1, sb)
                _esA.close()
                psS = _es.enter_context(tc.tile_pool(name="psS", bufs=3, space="PSUM"))
                psC = _es.enter_context(tc.tile_pool(name="psC", bufs=3, space="PSUM"))
                psB = _es.enter_context(tc.tile_pool(name="psB", bufs=2, space="PSUM"))

                # wo ring-reuses xts slots 0..7: DMA waits until A-v's last read
                wos = []
                for a in range(8):
                    wt_ = xw.tile([128, S], BF16, name=f"wo{a}", tag="xw")
                    nc.sync.dma_start(wt_[:], wo[a * 128:(a + 1) * 128, :])
                    wos.append(wt_)

                # strip-deferred normalization state: (pc, rec, h, t)
                pending = []

                def flush_pending():
                    while pending:
                        pcp_, recp_, hp_, tp2_ = pending.pop(0)
                        nc.vector.tensor_mul(
                            ctx_sb[hp_][:, tp2_ * 512:(tp2_ + 1) * 512],
                            pcp_[:], recp_[:])

                def emit_b(h, t):
                    flush_pending()
                    njt = 4 * t + 4
                    qr = qrot[h][:, t * 512:(t + 1) * 512]
                    pc = psC.tile([128, 512], F32, name=f"pc{h}{t}", tag="pc")
                    pm = psB.tile([128, 512], F32, name=f"pm{h}{t}", tag="pmb")
                    exs = []

                    def emit_front(jt):
                        # scoresT block + exp into SBUF (+ diagonal mask)
                        cut = 128 * (jt - 4 * t) if jt >= 4 * t else 0
                        pss = psS.tile([128, 512], F32,
                                       name=f"pss{h}{t}{jt}", tag="pss")
                        nc.tensor.matmul(pss[:, cut:512],
                                         krot[h][:, jt * 128:(jt + 1) * 128],
                                         qr[:, cut:512], start=True, stop=True,
                                         skip_group_check=True)
                        ex = w512.tile([128, 512], BF16,
                                       name=f"ex{h}{t}{jt}", tag="w512")
                        nc.scalar.activation(
                            ex[:, cut:512], pss[:, cut:512],
                            mybir.ActivationFunctionType.Exp, scale=SCALE)
                        if jt >= 4 * t:
                            nc.vector.tensor_mul(
                                ex[:, cut:cut + 128],
                                ex[:, cut:cut + 128], mask_sb[:])
                        exs.append((ex, cut))

                    def emit_back(jt):
                        ex, cut = exs[jt]
                        nc.tensor.matmul(pm[:, cut:512], ones128[:],
                                         ex[:, cut:512],
                                         start=(jt == 0), stop=(jt == njt - 1),
                                         skip_group_check=True)
                        nc.tensor.matmul(pc[:, cut:512],
                                         vsb[jt][:, h * 128:(h + 1) * 128],
                                         ex[:, cut:512],
                                         start=(jt == 0), stop=(jt == njt - 1),
                                         skip_group_check=True)

                    with nc.named_scope("B_attn"):
                        emit_front(0)
                        emit_front(1)
                        emit_front(2)
                        for jt in range(3, njt):
                            emit_front(jt)
                            emit_back(jt - 3)
                        emit_back(njt - 3)
                        emit_back(njt - 2)
                        emit_back(njt - 1)
                        rec = smp.tile([128, 512], BF16,
                                       name=f"rec{h}{t}", tag="rec")
                        nc.vector.reciprocal(rec[:], pm[:])
                        pending.append((pc, rec, h, t))

                def emit_c(t):
                    with nc.named_scope(f"C_out{t}"):
                        for ms in range(4):
                            for sbl in range(4):
                                sb = 4 * t + sbl
                                po = psC.tile([128, 512], F32,
                                              name=f"po{t}{sbl}{ms}", tag="pc")
                                for ht in range(HL):
                                    nc.tensor.matmul(
                                        po[:],
                                        ctx_sb[ht][:, sb * 128:(sb + 1) * 128],
                                        wos[ht][:, ms * 512:(ms + 1) * 512],
                                        start=(ht == 0), stop=(ht == HL - 1))
                                ot = cop.tile([128, 512], BF16,
                                              name=f"ot{t}{sbl}{ms}", tag="ot")
                                nc.scalar.copy(ot[:], po[:])
                                if t == 3:
                                    dst = outp3[ms // 2][sbl * 128:(sbl + 1) * 128,
                                                         (ms % 2) * 512:(ms % 2 + 1) * 512]
                                else:
                                    dst = outp_t[t][sbl * 128:(sbl + 1) * 128,
                                                    ms * 512:(ms + 1) * 512]
                                nc.sync.dma_start(dst, ot[:])
                            if t == 3 and ms % 2 == 1:
                                piece = ms // 2
                                nc.gpsimd.collective_compute(
                                    "ReduceScatter", mybir.AluOpType.add,
                                    replica_groups=[[0, 1], [2, 3], [4, 5], [6, 7]],
                                    ins=[outp3[piece][:]], outs=[rs3[piece][:]])
                                nc.gpsimd.dma_start(
                                    y[3][:, piece * 1024:(piece + 1) * 1024],
                                    rs3[piece][:])
                        if t < 3:
                            nc.gpsimd.collective_compute(
                                "ReduceScatter", mybir.AluOpType.add,
                                replica_groups=[[0, 1], [2, 3], [4, 5], [6, 7]],
                                ins=[outp_t[t][:]], outs=[rs_t[t][:]])
                            nc.gpsimd.dma_start(y[t], rs_t[t][:])

                # B strips with A-v blocks interleaved; C lags B by one strip
                for t in range(4):
                    for h in range(HL):
                        emit_b(h, t)
                    if t == 1:
                        flush_pending()
                        emit_c(0)
                    elif t == 2:
                        flush_pending()
                        emit_c(1)
                        emit_c(2)
                flush_pending()
                emit_c(3)

    _split_excess_waits(nc)
    return nc


# ---------------------------------------------------------------------------
# Host-side input prep / sharding
# ---------------------------------------------------------------------------

def _rope_tables():
    half = D // 2
    fraction = 2.0 * np.arange(half, dtype=np.float64) / D
    ts = MIN_WINDOW * (MAX_WINDOW / MIN_WINDOW) ** fraction
    ts = np.repeat(ts, 2)                              # [D]
    pos = np.arange(S, dtype=np.float64)
    sinusoid = pos[None, :] / ts[:, None]              # [D, S]
    cos = np.cos(sinusoid).astype(NP_BF16)
    sign = np.where(np.arange(D) % 2 == 1, 1.0, -1.0)
    sin = (np.sin(sinusoid) * sign[:, None]).astype(NP_BF16)
    return cos, sin


def _mask128():
    jj = np.arange(128)[:, None]
    ii = np.arange(128)[None, :]
    return (jj <= ii).astype(NP_BF16)



_CACHED = {}


def kernel(x, Wqkv, Wo):
    x = np.asarray(x, dtype=np.float32)
    Wqkv = np.asarray(Wqkv, dtype=np.float32)
    Wo = np.asarray(Wo, dtype=np.float32)

    cos, sin = _rope_tables()
    m128 = _mask128()

    in_maps = []
    for c in range(8):
        b, g = c // 2, c % 2
        hs = slice(g * HL, (g + 1) * HL)
        in_maps.append({
            "xt": np.ascontiguousarray(x[b].T).astype(NP_BF16),
            "wq": np.ascontiguousarray(
                Wqkv[:, 0, hs, :].reshape(M, HD)).astype(NP_BF16),
            "wk": np.ascontiguousarray(
                Wqkv[:, 1, hs, :].reshape(M, HD)).astype(NP_BF16),
            "wv": np.ascontiguousarray(
                Wqkv[:, 2, hs, :].reshape(M, HD)).astype(NP_BF16),
            "wo": np.ascontiguousarray(
                Wo[g * HD:(g + 1) * HD, :]).astype(NP_BF16),
            "cosT": cos, "sinT": sin, "mask128": m128,
        })

    if "nc" not in _CACHED:
        _CACHED["nc"] = build_kernel()
    nc = _CACHED["nc"]

    res = run_bass_kernel_spmd(nc, in_maps, core_ids=list(range(8)),
                               trace=os.environ.get("MHA_KERNEL_TRACE", "0") == "1")
    _CACHED["last_results"] = res

    out = np.empty((B, S, M), dtype=np.float32)
    for b in range(B):
        for half, r in ((0, res.results[2 * b]["y"]),
                        (256, res.results[2 * b + 1]["y"])):
            for t in range(4):
                out[b, t * 512 + half: t * 512 + half + 256] = \
                    np.asarray(r[t]).astype(np.float32)
    return out


if __name__ == "__main__":
    rng = np.random.default_rng(0)
    x = rng.standard_normal((B, S, M), dtype=np.float32)
    Wqkv = (rng.standard_normal((M, 3, H, D), dtype=np.float32) / math.sqrt(M)).astype(np.float32)
    Wo = (rng.standard_normal((H * D, M), dtype=np.float32) / math.sqrt(H * D)).astype(np.float32)
    out = kernel(x=x, Wqkv=Wqkv, Wo=Wo)
    print("kernel ran, out shape", out.shape, "mean", float(np.abs(out).mean()))
